# revision 40
# baseline (speedup 1.0000x reference)
"""Trainium2 Bass kernel for BasicCNN+LSTM (conv3x3+ReLU+GAP -> custom LSTM scan).

Self-contained: hardcodes shapes/sharding. Data-parallel over batch B=8 across
8 NeuronCores; each core processes one batch element end-to-end, the host
gathers the 8 [1,32] results.

Per-core device pipeline (per frame t of 24):
  - Conv packed M=128: each moving column = 8 vertically-adjacent output
    pixels x 16 filters (3 filter groups), K=90 = 3c x 3dx x 10 window rows.
    Host-prepacked stack [90, 1568] bf16, one DMA per frame.
  - 12 matmuls/frame (3 groups x 4 chunks of N=392), g-major, into
    [128, 2048] PSUM tiles (one per group, 4 bank-aligned chunk slots,
    bufs=2). M=128 triggers Fast Weight Load, making the per-matmul
    LDWEIGHTS ~4x cheaper.
  - One fused ReLU(+conv-bias)+GAP drain per group tile ([128, 4, 392],
    ScalarE activation / VectorE scalar_tensor_tensor alternating), accum_out
    columns are the complete per-group GAP sums (no extra reduce).
  - Scan step cost collapsed to ONE ScalarE sigmoid per step:
    tanh(a) = 2*sigmoid(2a) - 1 with the 2x folded into host-side weights,
    and the cell-path matmul's moving matrix carries a 2*I block so
    pg = [zW1, zW2, 2*zW3, 2*c_{t-1}] comes out of PE accumulation directly.
    h_{t-1} is recomputed as c_{t-1}*(2*sig(2c_{t-1})-1) instead of stored.
    Elementwise gate math runs on idle GPSIMD; DVE only does the 32x32 cell
    transpose. Gate PSUM is colocated in the just-drained chunk-3 region of
    each frame's g2 conv tile (overlap forces drain-before-gate ordering).
  - The reference's state-order swap bug is reproduced faithfully.
"""
import sys
if '/opt/trn_rl_repo' not in sys.path:
    sys.path.insert(0, '/opt/trn_rl_repo')

import numpy as np
import ml_dtypes

import concourse.bass as bass
import concourse.mybir as mybir
import concourse.tile as tile
from concourse.vector_clock import ScopedClock
from concourse.bass_utils import run_bass_kernel_spmd

# ---------------------------------------------------------------- constants
B, T, H, W, C, F, U = 8, 24, 112, 112, 3, 48, 32
P8 = 8             # pixels packed per moving column
WR = 10            # window rows per 8-pixel group
KP = 90            # stack partitions: 3c x 3dx x 10 window rows
M = 128            # 8 px x 16 filters
G = 3              # filter groups
NB = 14            # jb groups (112 rows / 8)
FREE = NB * W      # 1568 stack cols per frame
NCH = 392          # matmul chunk cols (4 chunks = FREE)

FP32 = mybir.dt.float32
BF16 = mybir.dt.bfloat16

LAST_RESULTS = None  # BassKernelResults of the most recent run (for test.py)

# ------------------------------------------------- TileContext drain patch
# The container's walrus rejects >1 semaphore wait per instruction; Tile's
# kernel-tail drain aggregates all end-of-kernel waits onto one Drain.
# Spread them across single-wait NOPs on the sync engine instead.
def _patched_drain_and_barrier(self, tick_clock, wait_clock):
    nc = self.nc
    probe = nc.sync.nop(nofuse=True, hint="tail_waits")
    wait_clock.add_sem_waits(probe.ins, ScopedClock({None: tick_clock.global_clock}))
    waits = list(probe.ins.sync_info.on_wait or [])
    if len(waits) > 1:
        probe.ins.sync_info.on_wait = waits[:1]
        for i in range(1, len(waits)):
            extra = nc.sync.nop(nofuse=True, hint=f"tail_waits_{i}")
            si = extra.ins.sync_info
            if si is None:
                extra.ins.sync_info = mybir.SyncInfo(on_wait=[waits[i]], on_update=[])
            else:
                si.on_wait = [waits[i]]
    nc.sync.drain()
    nc.all_engine_barrier()
    popped = nc._tile_sem_poison_stack.pop()
    assert popped is self._sem_poison
    nc.clear_and_free_semaphores(list(self.sems.allocated().values()))
    nc.all_engine_barrier()


tile.TileContext._drain_and_barrier = _patched_drain_and_barrier

# Same walrus restriction for regular instructions: spill extra sem waits
# onto preceding same-engine NOPs at commit time.
_orig_commit = tile.TileContext._commit_instruction


def _patched_commit(self, inst, *args, **kwargs):
    si = getattr(inst, 'sync_info', None)
    if si is not None and si.on_wait and len(si.on_wait) > 1 \
            and inst.engine != mybir.EngineType.Unassigned:
        waits = list(si.on_wait)
        si.on_wait = waits[-1:]
        for w in waits[:-1]:
            nop = mybir.InstNoOp(
                name=self.nc.get_next_instruction_name(),
                ins=[], outs=[], bass_is_fusable=False)
            nop.engine = inst.engine
            nop.sync_info = mybir.SyncInfo(on_wait=[w], on_update=[])
            _orig_commit(self, nop, *args, **kwargs)
    return _orig_commit(self, inst, *args, **kwargs)


tile.TileContext._commit_instruction = _patched_commit


# ------------------------------------------------------------- device code
def _build_bass(use_gbias=True, use_cbias=True):
    _build_bass.use_gbias = use_gbias
    _build_bass.use_cbias = use_cbias
    nc = bass.Bass('TRN2', target_bir_lowering=False, debug=False)

    xin = nc.dram_tensor('xin', [T, KP, FREE], BF16, kind='ExternalInput')
    smat_d = nc.dram_tensor('smat', [G, KP, M], BF16, kind='ExternalInput')
    cbias_d = nc.dram_tensor('cbias', [M, G], FP32, kind='ExternalInput')
    wfeat_d = nc.dram_tensor('wfeat', [M, G * 96], BF16, kind='ExternalInput')
    whidi_d = nc.dram_tensor('whidi', [M, M], BF16, kind='ExternalInput')
    gbias_d = nc.dram_tensor('gbias', [1, M], BF16, kind='ExternalInput')
    outh_d = nc.dram_tensor('outh', [1, U], FP32, kind='ExternalOutput')

    Relu = mybir.ActivationFunctionType.Relu
    Sigmoid = mybir.ActivationFunctionType.Sigmoid
    Amax = mybir.AluOpType.max
    Aadd = mybir.AluOpType.add
    Amult = mybir.AluOpType.mult
    Asub = mybir.AluOpType.subtract

    with tile.TileContext(nc) as tc:
        const = tc.alloc_tile_pool(name='const', bufs=1)
        state = tc.alloc_tile_pool(name='state', bufs=1)
        stackp = tc.alloc_tile_pool(name='stack', bufs=3)
        psum = tc.alloc_tile_pool(name='psum', bufs=2, space='PSUM')
        gs = tc.alloc_tile_pool(name='gs', bufs=6)
        zb_pool = tc.alloc_tile_pool(name='zb', bufs=6)
        ga_pool = tc.alloc_tile_pool(name='ga', bufs=4)
        tmp = tc.alloc_tile_pool(name='tmp', bufs=8)
        scra = tc.alloc_tile_pool(name='scra', bufs=2)
        scrb = tc.alloc_tile_pool(name='scrb', bufs=2)

        # constants (per-group stationaries in their own contiguous tiles:
        # a contiguous full-128-col bf16 weight load triggers FWL)
        smats = []
        for g in range(G):
            sc_g = const.tile([KP, M], BF16, tag=f'sc{g}', name=f'sc{g}')
            nc.sync.dma_start(sc_g[:], smat_d[g])
            smats.append(sc_g)
        cbias = const.tile([M, G], FP32, tag='cb')
        nc.sync.dma_start(cbias[:], cbias_d[:])
        wfeat = const.tile([M, G * 96], BF16, tag='wf')
        nc.sync.dma_start(wfeat[:], wfeat_d[:])
        whidi = const.tile([M, M], BF16, tag='wh')
        nc.sync.dma_start(whidi[:], whidi_d[:])
        if use_gbias:
            gbias = const.tile([M, M], BF16, tag='gb')
            nc.vector.memset(gbias[:], 0.0)
            nc.sync.dma_start(gbias[0:1, :], gbias_d[:])
            ones1 = const.tile([M, 65], BF16, tag='one1')
            nc.vector.memset(ones1[:], 0.0)
            nc.vector.memset(ones1[0:1, 0:1], 1.0)
        onesc = const.tile([1, U], FP32, tag='ones')
        nc.vector.memset(onesc[:], 1.0)
        zeros2k = const.tile([M, 2048], FP32, tag='z2k')
        nc.vector.memset(zeros2k[:], 0.0)

        # persistent scan state: cell in row 0 of a 32x32 bf16 block; DVE
        # 32x32 transpose yields its partition-form column for the next
        # step's cell matmul. Scan stationaries are padded to [128, 65] so
        # every matmul runs in the same (128, 128) PE tiling mode as the
        # conv (mode switches force PE drains).
        cellblk = state.tile([32, 32], BF16, tag='cellblk')
        cpblk = state.tile([128, 65], BF16, tag='cpblk')
        cellv = cellblk[0:1, 0:U]                       # current new_cell
        cell_part = cpblk[:]                            # col 0 = cell
        nc.vector.memset(cellblk[:], 0.0)
        nc.vector.memset(cpblk[:], 0.0)

        # per-frame accum bookkeeping: zcols[t] = per-group [128, 65] bf16
        # stationary tiles; g2tile[t] = the g2 conv psum tile (hosts the
        # colocated gate region; with the scan emitted after the NEXT
        # frame's g0 gen, this is the oldest pool-safe generation).
        zcols = [None] * T
        g2tile = [None] * T
        frames = [None] * T

        def get_frame(t):
            if frames[t] is None:
                rt = stackp.tile([128, FREE], BF16, tag='stk')
                nc.sync.dma_start(rt[0:KP, :], xin[t])
                frames[t] = rt
            return frames[t]

        def emit_conv(t, mid_hook=None):
            rt = get_frame(t)
            if t + 2 < T:
                get_frame(t + 2)  # prefetch
            accA = gs.tile([M, G], FP32, tag='accA')
            accB = gs.tile([M, G], FP32, tag='accB')
            for g in range(G):
                ps = psum.tile([M, 2048], FP32, tag='ps')
                for k in range(4):
                    nc.tensor.matmul(ps[:, 512 * k:512 * k + NCH],
                                     smats[g][:],
                                     rt[0:KP, NCH * k:NCH * k + NCH],
                                     start=True, stop=True)
                # split drain: ACT takes chunks 0-1 (can start mid-gen),
                # DVE chunks 2-3 -- halves the PSUM-recycle wall, so the
                # PE stops stalling on bank recycling.
                psv = ps.rearrange("p (s n) -> p s n", s=4)
                psvA = psv[:, 0:2, 0:NCH]
                psvB = psv[:, 2:4, 0:NCH]
                scr = scra.tile([M, FREE], BF16, tag='scrA')
                svA = scr.rearrange("p (s n) -> p s n", s=2)[:, :, 0:NCH]
                nc.scalar.activation(svA, psvA, Relu,
                                     bias=cbias[:, g:g + 1],
                                     accum_out=accA[:, g:g + 1])
                scr2 = scrb.tile([M, FREE], BF16, tag='scrB')
                svB = scr2.rearrange("p (s n) -> p s n", s=2)[:, :, 0:NCH]
                if _build_bass.use_cbias:
                    zv = zeros2k.rearrange("p (s n) -> p s n", s=2)[:, :, 0:NCH]
                    nc.vector.scalar_tensor_tensor(
                        out=svB, in0=psvB, scalar=cbias[:, g:g + 1],
                        in1=zv, op0=Aadd, op1=Amax,
                        accum_out=accB[:, g:g + 1])
                else:
                    nc.vector.tensor_scalar(
                        out=svB, in0=psvB, scalar1=0.0, scalar2=None,
                        op0=Amax, op1=Aadd, accum_out=accB[:, g:g + 1])
                if g == 1:
                    g2tile[t] = ps
            # bf16 conversion of the GAP sums on idle GPSIMD (as add-zero;
            # TensorTensor add is the known-ISA-legal Pool op). Each group's
            # stationary is a [128, 65] tile (col 0 live, rest zeroed) so
            # the scan matmuls stay in (128, 128) PE tiling mode.
            zlist = []
            for g in range(G):
                zb = zb_pool.tile([M, 65], BF16, tag=f'zb{g}')
                nc.gpsimd.memset(zb[:], 0.0)
                nc.gpsimd.tensor_add(zb[:, 0:1], accA[:, g:g + 1],
                                     accB[:, g:g + 1])
                zlist.append(zb)
            zcols[t] = zlist

        def emit_scan(t):
            # Gate preactivations in the retired chunk-3 region of the most
            # recent g2 conv tile (region overlap with the drain's read
            # range forces drain-before-gate ordering).
            host = g2tile[t + LAG - 1] if t + LAG - 1 < T else g2tile[T - 1]
            pg65 = host[0:65, 1536:1536 + M]
            pg65n = host[0:65, 1536:1536 + 96]
            # cell path first (start=True zeroes the region): moving carries
            # [Wh-blocks (x2 on tanh cols) | 2*I] so pg also gets 2*c_{t-1}.
            # Stationaries padded to [128, 65]: rows 1-64 of the out are
            # garbage in retired PSUM, only partition 0 is read.
            nc.tensor.matmul(pg65, cell_part, whidi[:],
                             start=True, stop=False, skip_group_check=True)
            for g in range(G):
                nc.tensor.matmul(pg65n, zcols[t][g][:],
                                 wfeat[:, 96 * g:96 * g + 96],
                                 start=False, stop=(g == G - 1 and
                                                    not _build_bass.use_gbias),
                                 skip_group_check=True)
            if _build_bass.use_gbias:
                nc.tensor.matmul(pg65, ones1[:], gbias[:],
                                 start=False, stop=True, skip_group_check=True)
            # ONE sigmoid: s = sig([zW1, zW2, 2*zW3, 2*c_{t-1}])
            sg = ga_pool.tile([1, M], FP32, tag='sg')
            nc.scalar.activation(sg[:], host[0:1, 1536:1536 + M], Sigmoid)
            # GPSIMD gate math (tanh(a) = 2*sig(2a)-1, via s+s-1):
            h4 = tmp.tile([1, U], FP32, tag='h4')
            nc.gpsimd.tensor_add(h4[:], sg[:, 96:128], sg[:, 96:128])
            nc.gpsimd.tensor_sub(h4[:], h4[:], onesc[:])
            hprev = tmp.tile([1, U], FP32, tag='hprev')
            nc.gpsimd.tensor_mul(hprev[:], h4[:], cellv)   # h_{t-1}
            t1 = tmp.tile([1, U], FP32, tag='t1')
            nc.gpsimd.tensor_mul(t1[:], sg[:, 0:U], hprev[:])
            t3 = tmp.tile([1, U], FP32, tag='t3')
            nc.gpsimd.tensor_add(t3[:], sg[:, 64:96], sg[:, 64:96])
            nc.gpsimd.tensor_sub(t3[:], t3[:], onesc[:])
            t2 = tmp.tile([1, U], FP32, tag='t2')
            nc.gpsimd.tensor_mul(t2[:], sg[:, U:2 * U], t3[:])
            nc.gpsimd.tensor_add(cellv, t1[:], t2[:])      # new_cell (bf16)
            if t < T - 1:
                nc.vector.transpose(cpblk[0:32, 0:32], cellblk[:])

        LAG = 4  # scan step t emitted after frame t+LAG-1's conv
        for t in range(T):
            if t >= LAG:
                emit_scan(t - LAG)
            emit_conv(t)
        for t in range(T - LAG, T):
            emit_scan(t)

        # tail: h_23 = c_23 * tanh(c_23) = c_23 * (2*sig(2*c_23) - 1)
        s4f = tmp.tile([1, U], FP32, tag='s4f')
        nc.scalar.activation(s4f[:], cellv, Sigmoid, scale=2.0)
        h4f = tmp.tile([1, U], FP32, tag='h4f')
        nc.gpsimd.tensor_add(h4f[:], s4f[:], s4f[:])
        nc.gpsimd.tensor_sub(h4f[:], h4f[:], onesc[:])
        hout = tmp.tile([1, U], FP32, tag='hout')
        nc.gpsimd.tensor_mul(hout[:], h4f[:], cellv)
        nc.sync.dma_start(outh_d[:], hout[:])

        for p in (scrb, scra, tmp, ga_pool, zb_pool, gs, psum, stackp,
                  state, const):
            p.release()

    return nc


# -------------------------------------------------------------- host prep
def _prep_inputs(x, conv_w, conv_b, W1, b1, W2, b2, W3, b3):
    x = np.asarray(x, np.float32)
    conv_w = np.asarray(conv_w, np.float32)
    conv_b = np.asarray(conv_b, np.float32)

    # stack [B, T, 90, 14*112]: p = (3c+dx)*10 + wr -> xpad[8jb+wr, x+dx, c]
    xp = np.zeros((B, T, H + 2, W + 2, C), np.float32)
    xp[:, :, 1:H + 1, 1:W + 1, :] = x
    xin3 = np.empty((B, T, KP, NB, W), np.float32)
    rows = 8 * np.arange(NB)
    for c in range(3):
        for dx in range(3):
            for wr in range(WR):
                p = (3 * c + dx) * WR + wr
                xin3[:, :, p] = np.moveaxis(
                    xp[:, :, rows + wr, dx:dx + W, c], 0, 2)
    xin3 = xin3.reshape(B, T, KP, FREE).astype(ml_dtypes.bfloat16)

    # stationary [3, 90, 128]: smat[g, p, 16i + fl] = w[wr-i, dx, c, 16g+fl]
    smat = np.zeros((G, KP, M), np.float32)
    for c in range(3):
        for dx in range(3):
            for i in range(P8):
                for dy in range(3):
                    wr = i + dy
                    p = (3 * c + dx) * WR + wr
                    for g in range(G):
                        smat[g, p, 16 * i:16 * i + 16] = \
                            conv_w[dy, dx, c, 16 * g:16 * g + 16]
    smat = smat.astype(ml_dtypes.bfloat16)

    # per-group conv bias, per partition 16i+fl
    cbias = np.empty((M, G), np.float32)
    for g in range(G):
        cbias[:, g] = np.tile(conv_b[16 * g:16 * g + 16], P8)

    # gate weights: cols [0:32]=W1, [32:64]=W2, [64:96]=2*W3 (tanh fold)
    Wcat = np.stack([np.asarray(W1, np.float32), np.asarray(W2, np.float32),
                     np.asarray(W3, np.float32)], axis=0)  # [3, 80, 32]
    scale = np.array([1.0, 1.0, 2.0], np.float32)
    # feat part: wfeat[128g+16i+fl -> wait: rows are (i, fl) of group g
    wfeat = np.zeros((M, G * 96), np.float32)
    for g in range(G):
        for gate in range(3):
            blk = Wcat[gate, 16 * g:16 * g + 16, :] * scale[gate] / float(H * W)
            # rows 16i+fl, cols 96g+32*gate+u : same 16-filter block per i
            wfeat[:, 96 * g + 32 * gate:96 * g + 32 * gate + 32] = \
                np.tile(blk, (P8, 1))
    wfeat = wfeat.astype(ml_dtypes.bfloat16)

    # hidden path + 2*I identity block: whidi [128, 128] (rows 32+ zero-pad
    # so the scan matmuls keep K=128 / (128,128) PE tiling mode)
    whidi = np.zeros((M, M), np.float32)
    for gate in range(3):
        whidi[0:U, 32 * gate:32 * gate + 32] = \
            Wcat[gate, F:F + U, :] * scale[gate]
    whidi[0:U, 96:128] = 2.0 * np.eye(U, dtype=np.float32)
    whidi = whidi.astype(ml_dtypes.bfloat16)

    gb = np.zeros((1, M), np.float32)
    gb[0, 0:32] = np.asarray(b1, np.float32)
    gb[0, 32:64] = np.asarray(b2, np.float32)
    gb[0, 64:96] = 2.0 * np.asarray(b3, np.float32)
    gbias = gb.astype(ml_dtypes.bfloat16)

    return xin3, smat, cbias, wfeat, whidi, gbias


# ------------------------------------------------------------------ kernel
def kernel(x, conv_w, conv_b, W1, b1, W2, b2, W3, b3, W4, b4):
    global LAST_RESULTS
    xin3, smat, cbias, wfeat, whidi, gbias = _prep_inputs(
        x, conv_w, conv_b, W1, b1, W2, b2, W3, b3)

    nc = _build_bass(use_gbias=bool(np.any(np.asarray(gbias, np.float32))))
    in_maps = [{
        'xin': np.ascontiguousarray(xin3[b]),
        'smat': smat,
        'cbias': cbias,
        'wfeat': wfeat,
        'whidi': whidi,
        'gbias': gbias,
    } for b in range(B)]

    res = run_bass_kernel_spmd(nc, in_maps, core_ids=list(range(B)))
    LAST_RESULTS = res
    out = np.stack([res.results[b]['outh'][0] for b in range(B)], axis=0)
    return out.astype(np.float32)


# revision 41
# speedup vs baseline: 1.3157x; 1.3157x over previous
"""Trainium2 Bass kernel for BasicCNN+LSTM (conv3x3+ReLU+GAP -> custom LSTM scan).

Self-contained: hardcodes shapes/sharding. Data-parallel over batch B=8 across
8 NeuronCores; each core processes one batch element end-to-end, the host
gathers the 8 [1,32] results.

Per-core device pipeline (per frame t of 24):
  - Conv packed M=128: each moving column = 8 vertically-adjacent output
    pixels x 16 filters (3 filter groups), K=90 = 3c x 3dx x 10 window rows.
    Host-prepacked stack [90, 1568] bf16, one DMA per frame.
  - 12 matmuls/frame (3 groups x 4 chunks of N=392), g-major, into
    [128, 2048] PSUM tiles (one per group, 4 bank-aligned chunk slots,
    bufs=2). M=128 triggers Fast Weight Load, making the per-matmul
    LDWEIGHTS ~4x cheaper.
  - One fused ReLU(+conv-bias)+GAP drain per group tile ([128, 4, 392],
    ScalarE activation / VectorE scalar_tensor_tensor alternating), accum_out
    columns are the complete per-group GAP sums (no extra reduce).
  - Scan step cost collapsed to ONE ScalarE sigmoid per step:
    tanh(a) = 2*sigmoid(2a) - 1 with the 2x folded into host-side weights,
    and the cell-path matmul's moving matrix carries a 2*I block so
    pg = [zW1, zW2, 2*zW3, 2*c_{t-1}] comes out of PE accumulation directly.
    h_{t-1} is recomputed as c_{t-1}*(2*sig(2c_{t-1})-1) instead of stored.
    Elementwise gate math runs on idle GPSIMD; DVE only does the 32x32 cell
    transpose. Gate PSUM is colocated in the just-drained chunk-3 region of
    each frame's g2 conv tile (overlap forces drain-before-gate ordering).
  - The reference's state-order swap bug is reproduced faithfully.
"""
import sys
if '/opt/trn_rl_repo' not in sys.path:
    sys.path.insert(0, '/opt/trn_rl_repo')

import numpy as np
import ml_dtypes

import concourse.bass as bass
import concourse.mybir as mybir
import concourse.tile as tile
from concourse.vector_clock import ScopedClock
from concourse.bass_utils import run_bass_kernel_spmd

# ---------------------------------------------------------------- constants
B, T, H, W, C, F, U = 8, 24, 112, 112, 3, 48, 32
P8 = 8             # pixels packed per moving column
WR = 10            # window rows per 8-pixel group
KP = 90            # stack partitions: 3c x 3dx x 10 window rows
M = 128            # 8 px x 16 filters
G = 3              # filter groups
NB = 14            # jb groups (112 rows / 8)
FREE = NB * W      # 1568 stack cols per frame
NCH = 392          # matmul chunk cols (4 chunks = FREE)

FP32 = mybir.dt.float32
BF16 = mybir.dt.bfloat16

LAST_RESULTS = None  # BassKernelResults of the most recent run (for test.py)

# ------------------------------------------------- TileContext drain patch
# The container's walrus rejects >1 semaphore wait per instruction; Tile's
# kernel-tail drain aggregates all end-of-kernel waits onto one Drain.
# Spread them across single-wait NOPs on the sync engine instead.
def _patched_drain_and_barrier(self, tick_clock, wait_clock):
    nc = self.nc
    probe = nc.sync.nop(nofuse=True, hint="tail_waits")
    wait_clock.add_sem_waits(probe.ins, ScopedClock({None: tick_clock.global_clock}))
    waits = list(probe.ins.sync_info.on_wait or [])
    if len(waits) > 1:
        probe.ins.sync_info.on_wait = waits[:1]
        for i in range(1, len(waits)):
            extra = nc.sync.nop(nofuse=True, hint=f"tail_waits_{i}")
            si = extra.ins.sync_info
            if si is None:
                extra.ins.sync_info = mybir.SyncInfo(on_wait=[waits[i]], on_update=[])
            else:
                si.on_wait = [waits[i]]
    nc.sync.drain()
    nc.all_engine_barrier()
    popped = nc._tile_sem_poison_stack.pop()
    assert popped is self._sem_poison
    nc.clear_and_free_semaphores(list(self.sems.allocated().values()))
    nc.all_engine_barrier()


tile.TileContext._drain_and_barrier = _patched_drain_and_barrier

# Same walrus restriction for regular instructions: spill extra sem waits
# onto preceding same-engine NOPs at commit time.
_orig_commit = tile.TileContext._commit_instruction


def _patched_commit(self, inst, *args, **kwargs):
    si = getattr(inst, 'sync_info', None)
    if si is not None and si.on_wait and len(si.on_wait) > 1 \
            and inst.engine != mybir.EngineType.Unassigned:
        waits = list(si.on_wait)
        si.on_wait = waits[-1:]
        for w in waits[:-1]:
            nop = mybir.InstNoOp(
                name=self.nc.get_next_instruction_name(),
                ins=[], outs=[], bass_is_fusable=False)
            nop.engine = inst.engine
            nop.sync_info = mybir.SyncInfo(on_wait=[w], on_update=[])
            _orig_commit(self, nop, *args, **kwargs)
    return _orig_commit(self, inst, *args, **kwargs)


tile.TileContext._commit_instruction = _patched_commit


# ------------------------------------------------------------- device code
def _build_bass(use_gbias=True, use_cbias=True):
    _build_bass.use_gbias = use_gbias
    _build_bass.use_cbias = use_cbias
    nc = bass.Bass('TRN2', target_bir_lowering=False, debug=False)

    xin = nc.dram_tensor('xin', [T, KP, FREE], BF16, kind='ExternalInput')
    smat_d = nc.dram_tensor('smat', [G, KP, M], BF16, kind='ExternalInput')
    cbias_d = nc.dram_tensor('cbias', [M, G], FP32, kind='ExternalInput')
    wfeat_d = nc.dram_tensor('wfeat', [M, G * 96], BF16, kind='ExternalInput')
    whidi_d = nc.dram_tensor('whidi', [M, M], BF16, kind='ExternalInput')
    gbias_d = nc.dram_tensor('gbias', [1, M], BF16, kind='ExternalInput')
    outh_d = nc.dram_tensor('outh', [1, U], FP32, kind='ExternalOutput')

    Relu = mybir.ActivationFunctionType.Relu
    Sigmoid = mybir.ActivationFunctionType.Sigmoid
    Amax = mybir.AluOpType.max
    Aadd = mybir.AluOpType.add
    Amult = mybir.AluOpType.mult
    Asub = mybir.AluOpType.subtract

    with tile.TileContext(nc) as tc:
        const = tc.alloc_tile_pool(name='const', bufs=1)
        state = tc.alloc_tile_pool(name='state', bufs=1)
        stackp = tc.alloc_tile_pool(name='stack', bufs=3)
        psum = tc.alloc_tile_pool(name='psum', bufs=3, space='PSUM')
        spsum = tc.alloc_tile_pool(name='spsum', bufs=2, space='PSUM')
        gs = tc.alloc_tile_pool(name='gs', bufs=6)
        zb_pool = tc.alloc_tile_pool(name='zb', bufs=6)
        ga_pool = tc.alloc_tile_pool(name='ga', bufs=4)
        tmp = tc.alloc_tile_pool(name='tmp', bufs=8)
        scra = tc.alloc_tile_pool(name='scra', bufs=2)
        scrb = tc.alloc_tile_pool(name='scrb', bufs=2)

        # constants (per-group stationaries in their own contiguous tiles:
        # a contiguous full-128-col bf16 weight load triggers FWL)
        smats = []
        for g in range(G):
            sc_g = const.tile([KP, M], BF16, tag=f'sc{g}', name=f'sc{g}')
            nc.sync.dma_start(sc_g[:], smat_d[g])
            smats.append(sc_g)
        cbias = const.tile([M, G], FP32, tag='cb')
        nc.sync.dma_start(cbias[:], cbias_d[:])
        wfeat = const.tile([M, G * 96], BF16, tag='wf')
        nc.sync.dma_start(wfeat[:], wfeat_d[:])
        whidi = const.tile([M, M], BF16, tag='wh')
        nc.sync.dma_start(whidi[:], whidi_d[:])
        if use_gbias:
            gbias = const.tile([M, M], BF16, tag='gb')
            nc.vector.memset(gbias[:], 0.0)
            nc.sync.dma_start(gbias[0:1, :], gbias_d[:])
            ones1 = const.tile([M, 65], BF16, tag='one1')
            nc.vector.memset(ones1[:], 0.0)
            nc.vector.memset(ones1[0:1, 0:1], 1.0)
        onesc = const.tile([1, U], FP32, tag='ones')
        nc.vector.memset(onesc[:], 1.0)
        zeros2k = const.tile([M, 2048], FP32, tag='z2k')
        nc.vector.memset(zeros2k[:], 0.0)

        # persistent scan state: cell in row 0 of a 32x32 bf16 block; DVE
        # 32x32 transpose yields its partition-form column for the next
        # step's cell matmul. Scan stationaries are padded to [128, 65] so
        # every matmul runs in the same (128, 128) PE tiling mode as the
        # conv (mode switches force PE drains).
        cellblk = state.tile([32, 32], BF16, tag='cellblk')
        cpblk = state.tile([128, 65], BF16, tag='cpblk')
        cellv = cellblk[0:1, 0:U]                       # current new_cell
        cell_part = cpblk[:]                            # col 0 = cell
        nc.vector.memset(cellblk[:], 0.0)
        nc.vector.memset(cpblk[:], 0.0)

        # per-frame accum bookkeeping: zcols[t] = per-group [128, 65] bf16
        # stationary tiles (GAP sums, the scan matmuls' stationaries).
        zcols = [None] * T
        frames = [None] * T

        def get_frame(t):
            if frames[t] is None:
                rt = stackp.tile([128, FREE], BF16, tag='stk')
                nc.sync.dma_start(rt[0:KP, :], xin[t])
                frames[t] = rt
            return frames[t]

        def emit_conv(t, mid_hook=None):
            rt = get_frame(t)
            if t + 2 < T:
                get_frame(t + 2)  # prefetch
            accA = gs.tile([M, G], FP32, tag='accA')
            accB = gs.tile([M, G], FP32, tag='accB')
            # 6 half-group gens of [128, 1024] (2 bank-aligned chunks),
            # bufs=3: more generations in flight so the mm->drain->recycle
            # loop latency stops pacing the PE. One drain per gen; the two
            # gens of each filter group go to different engines, accum
            # columns merged in the gpsimd bf16 convert.
            for g in range(G):
                for h in range(2):
                    ps = psum.tile([M, 1024], FP32, tag='ps')
                    for kk in range(2):
                        k = 2 * h + kk
                        nc.tensor.matmul(ps[:, 512 * kk:512 * kk + NCH],
                                         smats[g][:],
                                         rt[0:KP, NCH * k:NCH * k + NCH],
                                         start=True, stop=True)
                    psv = ps.rearrange("p (s n) -> p s n", s=2)[:, :, 0:NCH]
                    use_act = (2 * g + h + t) % 2 == 0
                    if use_act:
                        scr = scra.tile([M, FREE], BF16, tag='scrA')
                        sv = scr.rearrange("p (s n) -> p s n", s=2)[:, :, 0:NCH]
                        nc.scalar.activation(sv, psv, Relu,
                                             bias=cbias[:, g:g + 1],
                                             accum_out=accA[:, g:g + 1])
                    else:
                        scr = scrb.tile([M, FREE], BF16, tag='scrB')
                        sv = scr.rearrange("p (s n) -> p s n", s=2)[:, :, 0:NCH]
                        if _build_bass.use_cbias:
                            zv = zeros2k.rearrange(
                                "p (s n) -> p s n", s=2)[:, :, 0:NCH]
                            nc.vector.scalar_tensor_tensor(
                                out=sv, in0=psv, scalar=cbias[:, g:g + 1],
                                in1=zv, op0=Aadd, op1=Amax,
                                accum_out=accB[:, g:g + 1])
                        else:
                            nc.vector.tensor_scalar(
                                out=sv, in0=psv, scalar1=0.0, scalar2=None,
                                op0=Amax, op1=Aadd,
                                accum_out=accB[:, g:g + 1])
            # bf16 conversion of the GAP sums on idle GPSIMD (as add-zero;
            # TensorTensor add is the known-ISA-legal Pool op). Each group's
            # stationary is a [128, 65] tile (col 0 live, rest zeroed) so
            # the scan matmuls stay in (128, 128) PE tiling mode.
            zlist = []
            for g in range(G):
                zb = zb_pool.tile([M, 65], BF16, tag=f'zb{g}')
                nc.gpsimd.memset(zb[:], 0.0)
                nc.gpsimd.tensor_add(zb[:, 0:1], accA[:, g:g + 1],
                                     accB[:, g:g + 1])
                zlist.append(zb)
            zcols[t] = zlist

        def emit_scan(t):
            # Gate preactivations in the retired chunk-3 region of the most
            # recent g2 conv tile (region overlap with the drain's read
            # range forces drain-before-gate ordering).
            sp = spsum.tile([M, M], FP32, tag='sp')
            pg65 = sp[0:65, 0:M]
            pg65n = sp[0:65, 0:96]
            # cell path first (start=True zeroes the region): moving carries
            # [Wh-blocks (x2 on tanh cols) | 2*I] so pg also gets 2*c_{t-1}.
            # Stationaries padded to [128, 65]: rows 1-64 of the out are
            # garbage in retired PSUM, only partition 0 is read.
            nc.tensor.matmul(pg65, cell_part, whidi[:],
                             start=True, stop=False, skip_group_check=True)
            for g in range(G):
                nc.tensor.matmul(pg65n, zcols[t][g][:],
                                 wfeat[:, 96 * g:96 * g + 96],
                                 start=False, stop=(g == G - 1 and
                                                    not _build_bass.use_gbias),
                                 skip_group_check=True)
            if _build_bass.use_gbias:
                nc.tensor.matmul(pg65, ones1[:], gbias[:],
                                 start=False, stop=True, skip_group_check=True)
            # ONE sigmoid: s = sig([zW1, zW2, 2*zW3, 2*c_{t-1}])
            sg = ga_pool.tile([1, M], FP32, tag='sg')
            nc.scalar.activation(sg[:], sp[0:1, 0:M], Sigmoid)
            # GPSIMD gate math (tanh(a) = 2*sig(2a)-1, via s+s-1):
            h4 = tmp.tile([1, U], FP32, tag='h4')
            nc.gpsimd.tensor_add(h4[:], sg[:, 96:128], sg[:, 96:128])
            nc.gpsimd.tensor_sub(h4[:], h4[:], onesc[:])
            hprev = tmp.tile([1, U], FP32, tag='hprev')
            nc.gpsimd.tensor_mul(hprev[:], h4[:], cellv)   # h_{t-1}
            t1 = tmp.tile([1, U], FP32, tag='t1')
            nc.gpsimd.tensor_mul(t1[:], sg[:, 0:U], hprev[:])
            t3 = tmp.tile([1, U], FP32, tag='t3')
            nc.gpsimd.tensor_add(t3[:], sg[:, 64:96], sg[:, 64:96])
            nc.gpsimd.tensor_sub(t3[:], t3[:], onesc[:])
            t2 = tmp.tile([1, U], FP32, tag='t2')
            nc.gpsimd.tensor_mul(t2[:], sg[:, U:2 * U], t3[:])
            nc.gpsimd.tensor_add(cellv, t1[:], t2[:])      # new_cell (bf16)
            if t < T - 1:
                nc.vector.transpose(cpblk[0:32, 0:32], cellblk[:])

        LAG = 4  # scan step t emitted after frame t+LAG-1's conv
        for t in range(T):
            if t >= LAG:
                emit_scan(t - LAG)
            emit_conv(t)
        for t in range(T - LAG, T):
            emit_scan(t)

        # tail: h_23 = c_23 * tanh(c_23) = c_23 * (2*sig(2*c_23) - 1)
        s4f = tmp.tile([1, U], FP32, tag='s4f')
        nc.scalar.activation(s4f[:], cellv, Sigmoid, scale=2.0)
        h4f = tmp.tile([1, U], FP32, tag='h4f')
        nc.gpsimd.tensor_add(h4f[:], s4f[:], s4f[:])
        nc.gpsimd.tensor_sub(h4f[:], h4f[:], onesc[:])
        hout = tmp.tile([1, U], FP32, tag='hout')
        nc.gpsimd.tensor_mul(hout[:], h4f[:], cellv)
        nc.sync.dma_start(outh_d[:], hout[:])

        for p in (scrb, scra, tmp, ga_pool, zb_pool, gs, spsum, psum,
                  stackp, state, const):
            p.release()

    return nc


# -------------------------------------------------------------- host prep
def _prep_inputs(x, conv_w, conv_b, W1, b1, W2, b2, W3, b3):
    x = np.asarray(x, np.float32)
    conv_w = np.asarray(conv_w, np.float32)
    conv_b = np.asarray(conv_b, np.float32)

    # stack [B, T, 90, 14*112]: p = (3c+dx)*10 + wr -> xpad[8jb+wr, x+dx, c]
    xp = np.zeros((B, T, H + 2, W + 2, C), np.float32)
    xp[:, :, 1:H + 1, 1:W + 1, :] = x
    xin3 = np.empty((B, T, KP, NB, W), np.float32)
    rows = 8 * np.arange(NB)
    for c in range(3):
        for dx in range(3):
            for wr in range(WR):
                p = (3 * c + dx) * WR + wr
                xin3[:, :, p] = np.moveaxis(
                    xp[:, :, rows + wr, dx:dx + W, c], 0, 2)
    xin3 = xin3.reshape(B, T, KP, FREE).astype(ml_dtypes.bfloat16)

    # stationary [3, 90, 128]: smat[g, p, 16i + fl] = w[wr-i, dx, c, 16g+fl]
    smat = np.zeros((G, KP, M), np.float32)
    for c in range(3):
        for dx in range(3):
            for i in range(P8):
                for dy in range(3):
                    wr = i + dy
                    p = (3 * c + dx) * WR + wr
                    for g in range(G):
                        smat[g, p, 16 * i:16 * i + 16] = \
                            conv_w[dy, dx, c, 16 * g:16 * g + 16]
    smat = smat.astype(ml_dtypes.bfloat16)

    # per-group conv bias, per partition 16i+fl
    cbias = np.empty((M, G), np.float32)
    for g in range(G):
        cbias[:, g] = np.tile(conv_b[16 * g:16 * g + 16], P8)

    # gate weights: cols [0:32]=W1, [32:64]=W2, [64:96]=2*W3 (tanh fold)
    Wcat = np.stack([np.asarray(W1, np.float32), np.asarray(W2, np.float32),
                     np.asarray(W3, np.float32)], axis=0)  # [3, 80, 32]
    scale = np.array([1.0, 1.0, 2.0], np.float32)
    # feat part: wfeat[128g+16i+fl -> wait: rows are (i, fl) of group g
    wfeat = np.zeros((M, G * 96), np.float32)
    for g in range(G):
        for gate in range(3):
            blk = Wcat[gate, 16 * g:16 * g + 16, :] * scale[gate] / float(H * W)
            # rows 16i+fl, cols 96g+32*gate+u : same 16-filter block per i
            wfeat[:, 96 * g + 32 * gate:96 * g + 32 * gate + 32] = \
                np.tile(blk, (P8, 1))
    wfeat = wfeat.astype(ml_dtypes.bfloat16)

    # hidden path + 2*I identity block: whidi [128, 128] (rows 32+ zero-pad
    # so the scan matmuls keep K=128 / (128,128) PE tiling mode)
    whidi = np.zeros((M, M), np.float32)
    for gate in range(3):
        whidi[0:U, 32 * gate:32 * gate + 32] = \
            Wcat[gate, F:F + U, :] * scale[gate]
    whidi[0:U, 96:128] = 2.0 * np.eye(U, dtype=np.float32)
    whidi = whidi.astype(ml_dtypes.bfloat16)

    gb = np.zeros((1, M), np.float32)
    gb[0, 0:32] = np.asarray(b1, np.float32)
    gb[0, 32:64] = np.asarray(b2, np.float32)
    gb[0, 64:96] = 2.0 * np.asarray(b3, np.float32)
    gbias = gb.astype(ml_dtypes.bfloat16)

    return xin3, smat, cbias, wfeat, whidi, gbias


# ------------------------------------------------------------------ kernel
def kernel(x, conv_w, conv_b, W1, b1, W2, b2, W3, b3, W4, b4):
    global LAST_RESULTS
    xin3, smat, cbias, wfeat, whidi, gbias = _prep_inputs(
        x, conv_w, conv_b, W1, b1, W2, b2, W3, b3)

    nc = _build_bass(
        use_gbias=bool(np.any(np.asarray(gbias, np.float32))),
        use_cbias=bool(np.any(np.asarray(conv_b, np.float32))))
    in_maps = [{
        'xin': np.ascontiguousarray(xin3[b]),
        'smat': smat,
        'cbias': cbias,
        'wfeat': wfeat,
        'whidi': whidi,
        'gbias': gbias,
    } for b in range(B)]

    res = run_bass_kernel_spmd(nc, in_maps, core_ids=list(range(B)))
    LAST_RESULTS = res
    out = np.stack([res.results[b]['outh'][0] for b in range(B)], axis=0)
    return out.astype(np.float32)


# revision 42
# speedup vs baseline: 1.3230x; 1.0055x over previous
"""Trainium2 Bass kernel for BasicCNN+LSTM (conv3x3+ReLU+GAP -> custom LSTM scan).

Self-contained: hardcodes shapes/sharding. Data-parallel over batch B=8 across
8 NeuronCores; each core processes one batch element end-to-end, the host
gathers the 8 [1,32] results.

Per-core device pipeline (per frame t of 24):
  - Conv packed M=128: each moving column = 8 vertically-adjacent output
    pixels x 16 filters (3 filter groups), K=90 = 3c x 3dx x 10 window rows.
    Host-prepacked stack [90, 1568] bf16, one DMA per frame.
  - 12 matmuls/frame (3 groups x 4 chunks of N=392), g-major, into
    [128, 2048] PSUM tiles (one per group, 4 bank-aligned chunk slots,
    bufs=2). M=128 triggers Fast Weight Load, making the per-matmul
    LDWEIGHTS ~4x cheaper.
  - One fused ReLU(+conv-bias)+GAP drain per group tile ([128, 4, 392],
    ScalarE activation / VectorE scalar_tensor_tensor alternating), accum_out
    columns are the complete per-group GAP sums (no extra reduce).
  - Scan step cost collapsed to ONE ScalarE sigmoid per step:
    tanh(a) = 2*sigmoid(2a) - 1 with the 2x folded into host-side weights,
    and the cell-path matmul's moving matrix carries a 2*I block so
    pg = [zW1, zW2, 2*zW3, 2*c_{t-1}] comes out of PE accumulation directly.
    h_{t-1} is recomputed as c_{t-1}*(2*sig(2c_{t-1})-1) instead of stored.
    Elementwise gate math runs on idle GPSIMD; DVE only does the 32x32 cell
    transpose. Gate PSUM is colocated in the just-drained chunk-3 region of
    each frame's g2 conv tile (overlap forces drain-before-gate ordering).
  - The reference's state-order swap bug is reproduced faithfully.
"""
import sys
if '/opt/trn_rl_repo' not in sys.path:
    sys.path.insert(0, '/opt/trn_rl_repo')

import numpy as np
import ml_dtypes

import concourse.bass as bass
import concourse.mybir as mybir
import concourse.tile as tile
from concourse.vector_clock import ScopedClock
from concourse.bass_utils import run_bass_kernel_spmd

# ---------------------------------------------------------------- constants
B, T, H, W, C, F, U = 8, 24, 112, 112, 3, 48, 32
P8 = 8             # pixels packed per moving column
WR = 10            # window rows per 8-pixel group
KP = 90            # stack partitions: 3c x 3dx x 10 window rows
M = 128            # 8 px x 16 filters
G = 3              # filter groups
NB = 14            # jb groups (112 rows / 8)
FREE = NB * W      # 1568 stack cols per frame
NCH = 392          # matmul chunk cols (4 chunks = FREE)

FP32 = mybir.dt.float32
BF16 = mybir.dt.bfloat16
FP8 = mybir.dt.float8e4

LAST_RESULTS = None  # BassKernelResults of the most recent run (for test.py)

# ------------------------------------------------- TileContext drain patch
# The container's walrus rejects >1 semaphore wait per instruction; Tile's
# kernel-tail drain aggregates all end-of-kernel waits onto one Drain.
# Spread them across single-wait NOPs on the sync engine instead.
def _patched_drain_and_barrier(self, tick_clock, wait_clock):
    nc = self.nc
    probe = nc.sync.nop(nofuse=True, hint="tail_waits")
    wait_clock.add_sem_waits(probe.ins, ScopedClock({None: tick_clock.global_clock}))
    waits = list(probe.ins.sync_info.on_wait or [])
    if len(waits) > 1:
        probe.ins.sync_info.on_wait = waits[:1]
        for i in range(1, len(waits)):
            extra = nc.sync.nop(nofuse=True, hint=f"tail_waits_{i}")
            si = extra.ins.sync_info
            if si is None:
                extra.ins.sync_info = mybir.SyncInfo(on_wait=[waits[i]], on_update=[])
            else:
                si.on_wait = [waits[i]]
    nc.sync.drain()
    nc.all_engine_barrier()
    popped = nc._tile_sem_poison_stack.pop()
    assert popped is self._sem_poison
    nc.clear_and_free_semaphores(list(self.sems.allocated().values()))
    nc.all_engine_barrier()


tile.TileContext._drain_and_barrier = _patched_drain_and_barrier

# Same walrus restriction for regular instructions: spill extra sem waits
# onto preceding same-engine NOPs at commit time.
_orig_commit = tile.TileContext._commit_instruction


def _patched_commit(self, inst, *args, **kwargs):
    si = getattr(inst, 'sync_info', None)
    if si is not None and si.on_wait and len(si.on_wait) > 1 \
            and inst.engine != mybir.EngineType.Unassigned:
        waits = list(si.on_wait)
        si.on_wait = waits[-1:]
        for w in waits[:-1]:
            nop = mybir.InstNoOp(
                name=self.nc.get_next_instruction_name(),
                ins=[], outs=[], bass_is_fusable=False)
            nop.engine = inst.engine
            nop.sync_info = mybir.SyncInfo(on_wait=[w], on_update=[])
            _orig_commit(self, nop, *args, **kwargs)
    return _orig_commit(self, inst, *args, **kwargs)


tile.TileContext._commit_instruction = _patched_commit


# ------------------------------------------------------------- device code
def _build_bass(use_gbias=True, use_cbias=True):
    _build_bass.use_gbias = use_gbias
    _build_bass.use_cbias = use_cbias
    nc = bass.Bass('TRN2', target_bir_lowering=False, debug=False)

    xin = nc.dram_tensor('xin', [T, KP, FREE], FP8, kind='ExternalInput')
    smat_d = nc.dram_tensor('smat', [G, KP, M], BF16, kind='ExternalInput')
    cbias_d = nc.dram_tensor('cbias', [M, G], FP32, kind='ExternalInput')
    wfeat_d = nc.dram_tensor('wfeat', [M, G * 96], BF16, kind='ExternalInput')
    whidi_d = nc.dram_tensor('whidi', [M, M], BF16, kind='ExternalInput')
    gbias_d = nc.dram_tensor('gbias', [1, M], BF16, kind='ExternalInput')
    outh_d = nc.dram_tensor('outh', [1, U], FP32, kind='ExternalOutput')

    Relu = mybir.ActivationFunctionType.Relu
    Sigmoid = mybir.ActivationFunctionType.Sigmoid
    Amax = mybir.AluOpType.max
    Aadd = mybir.AluOpType.add
    Amult = mybir.AluOpType.mult
    Asub = mybir.AluOpType.subtract

    with tile.TileContext(nc) as tc:
        const = tc.alloc_tile_pool(name='const', bufs=1)
        state = tc.alloc_tile_pool(name='state', bufs=1)
        stackp = tc.alloc_tile_pool(name='stack', bufs=3)
        psum = tc.alloc_tile_pool(name='psum', bufs=3, space='PSUM')
        spsum = tc.alloc_tile_pool(name='spsum', bufs=2, space='PSUM')
        gs = tc.alloc_tile_pool(name='gs', bufs=6)
        zb_pool = tc.alloc_tile_pool(name='zb', bufs=6)
        ga_pool = tc.alloc_tile_pool(name='ga', bufs=4)
        tmp = tc.alloc_tile_pool(name='tmp', bufs=8)
        scra = tc.alloc_tile_pool(name='scra', bufs=2)
        scrb = tc.alloc_tile_pool(name='scrb', bufs=2)

        # constants (per-group stationaries in their own contiguous tiles:
        # a contiguous full-128-col bf16 weight load triggers FWL)
        smats = []
        for g in range(G):
            sc_g = const.tile([KP, M], BF16, tag=f'sc{g}', name=f'sc{g}')
            nc.sync.dma_start(sc_g[:], smat_d[g])
            smats.append(sc_g)
        cbias = const.tile([M, G], FP32, tag='cb')
        nc.sync.dma_start(cbias[:], cbias_d[:])
        wfeat = const.tile([M, G * 96], BF16, tag='wf')
        nc.sync.dma_start(wfeat[:], wfeat_d[:])
        whidi = const.tile([M, M], BF16, tag='wh')
        nc.sync.dma_start(whidi[:], whidi_d[:])
        if use_gbias:
            gbias = const.tile([M, M], BF16, tag='gb')
            nc.vector.memset(gbias[:], 0.0)
            nc.sync.dma_start(gbias[0:1, :], gbias_d[:])
            ones1 = const.tile([M, 65], BF16, tag='one1')
            nc.vector.memset(ones1[:], 0.0)
            nc.vector.memset(ones1[0:1, 0:1], 1.0)
        onesc = const.tile([1, U], FP32, tag='ones')
        nc.vector.memset(onesc[:], 1.0)
        zeros2k = const.tile([M, 2048], FP32, tag='z2k')
        nc.vector.memset(zeros2k[:], 0.0)

        # persistent scan state: cell in row 0 of a 32x32 bf16 block; DVE
        # 32x32 transpose yields its partition-form column for the next
        # step's cell matmul. Scan stationaries are padded to [128, 65] so
        # every matmul runs in the same (128, 128) PE tiling mode as the
        # conv (mode switches force PE drains).
        cellblk = state.tile([32, 32], BF16, tag='cellblk')
        cpblk = state.tile([128, 65], BF16, tag='cpblk')
        cellv = cellblk[0:1, 0:U]                       # current new_cell
        cell_part = cpblk[:]                            # col 0 = cell
        nc.vector.memset(cellblk[:], 0.0)
        nc.vector.memset(cpblk[:], 0.0)

        # per-frame accum bookkeeping: zcols[t] = per-group [128, 65] bf16
        # stationary tiles (GAP sums, the scan matmuls' stationaries).
        zcols = [None] * T
        frames = [None] * T

        def get_frame(t):
            if frames[t] is None:
                rt = stackp.tile([128, FREE], FP8, tag='stk')
                nc.sync.dma_start(rt[0:KP, :], xin[t])
                frames[t] = rt
            return frames[t]

        def emit_conv(t, mid_hook=None):
            rt = get_frame(t)
            if t + 2 < T:
                get_frame(t + 2)  # prefetch
            accA = gs.tile([M, G], FP32, tag='accA')
            accB = gs.tile([M, G], FP32, tag='accB')
            # 6 half-group gens of [128, 1024] (2 bank-aligned chunks),
            # bufs=3: more generations in flight so the mm->drain->recycle
            # loop latency stops pacing the PE. One drain per gen; the two
            # gens of each filter group go to different engines, accum
            # columns merged in the gpsimd bf16 convert.
            for g in range(G):
                for h in range(2):
                    ps = psum.tile([M, 1024], FP32, tag='ps')
                    for kk in range(2):
                        k = 2 * h + kk
                        nc.tensor.matmul(ps[:, 512 * kk:512 * kk + NCH],
                                         smats[g][:],
                                         rt[0:KP, NCH * k:NCH * k + NCH],
                                         start=True, stop=True)
                    psv = ps.rearrange("p (s n) -> p s n", s=2)[:, :, 0:NCH]
                    use_act = (2 * g + h + t) % 2 == 0
                    if use_act:
                        scr = scra.tile([M, FREE], BF16, tag='scrA')
                        sv = scr.rearrange("p (s n) -> p s n", s=2)[:, :, 0:NCH]
                        nc.scalar.activation(sv, psv, Relu,
                                             bias=cbias[:, g:g + 1],
                                             accum_out=accA[:, g:g + 1])
                    else:
                        scr = scrb.tile([M, FREE], BF16, tag='scrB')
                        sv = scr.rearrange("p (s n) -> p s n", s=2)[:, :, 0:NCH]
                        if _build_bass.use_cbias:
                            zv = zeros2k.rearrange(
                                "p (s n) -> p s n", s=2)[:, :, 0:NCH]
                            nc.vector.scalar_tensor_tensor(
                                out=sv, in0=psv, scalar=cbias[:, g:g + 1],
                                in1=zv, op0=Aadd, op1=Amax,
                                accum_out=accB[:, g:g + 1])
                        else:
                            nc.vector.tensor_scalar(
                                out=sv, in0=psv, scalar1=0.0, scalar2=None,
                                op0=Amax, op1=Aadd,
                                accum_out=accB[:, g:g + 1])
            # bf16 conversion of the GAP sums on idle GPSIMD (as add-zero;
            # TensorTensor add is the known-ISA-legal Pool op). Each group's
            # stationary is a [128, 65] tile (col 0 live, rest zeroed) so
            # the scan matmuls stay in (128, 128) PE tiling mode.
            zlist = []
            for g in range(G):
                zb = zb_pool.tile([M, 65], BF16, tag=f'zb{g}')
                nc.gpsimd.memset(zb[:], 0.0)
                nc.gpsimd.tensor_add(zb[:, 0:1], accA[:, g:g + 1],
                                     accB[:, g:g + 1])
                zlist.append(zb)
            zcols[t] = zlist

        def emit_scan(t):
            # Gate preactivations in the retired chunk-3 region of the most
            # recent g2 conv tile (region overlap with the drain's read
            # range forces drain-before-gate ordering).
            sp = spsum.tile([M, M], FP32, tag='sp')
            pg65 = sp[0:65, 0:M]
            pg65n = sp[0:65, 0:96]
            # cell path first (start=True zeroes the region): moving carries
            # [Wh-blocks (x2 on tanh cols) | 2*I] so pg also gets 2*c_{t-1}.
            # Stationaries padded to [128, 65]: rows 1-64 of the out are
            # garbage in retired PSUM, only partition 0 is read.
            nc.tensor.matmul(pg65, cell_part, whidi[:],
                             start=True, stop=False, skip_group_check=True)
            for g in range(G):
                nc.tensor.matmul(pg65n, zcols[t][g][:],
                                 wfeat[:, 96 * g:96 * g + 96],
                                 start=False, stop=(g == G - 1 and
                                                    not _build_bass.use_gbias),
                                 skip_group_check=True)
            if _build_bass.use_gbias:
                nc.tensor.matmul(pg65, ones1[:], gbias[:],
                                 start=False, stop=True, skip_group_check=True)
            # ONE sigmoid: s = sig([zW1, zW2, 2*zW3, 2*c_{t-1}])
            sg = ga_pool.tile([1, M], FP32, tag='sg')
            nc.scalar.activation(sg[:], sp[0:1, 0:M], Sigmoid)
            # GPSIMD gate math (tanh(a) = 2*sig(2a)-1, via s+s-1):
            h4 = tmp.tile([1, U], FP32, tag='h4')
            nc.gpsimd.tensor_add(h4[:], sg[:, 96:128], sg[:, 96:128])
            nc.gpsimd.tensor_sub(h4[:], h4[:], onesc[:])
            hprev = tmp.tile([1, U], FP32, tag='hprev')
            nc.gpsimd.tensor_mul(hprev[:], h4[:], cellv)   # h_{t-1}
            t1 = tmp.tile([1, U], FP32, tag='t1')
            nc.gpsimd.tensor_mul(t1[:], sg[:, 0:U], hprev[:])
            t3 = tmp.tile([1, U], FP32, tag='t3')
            nc.gpsimd.tensor_add(t3[:], sg[:, 64:96], sg[:, 64:96])
            nc.gpsimd.tensor_sub(t3[:], t3[:], onesc[:])
            t2 = tmp.tile([1, U], FP32, tag='t2')
            nc.gpsimd.tensor_mul(t2[:], sg[:, U:2 * U], t3[:])
            nc.gpsimd.tensor_add(cellv, t1[:], t2[:])      # new_cell (bf16)
            if t < T - 1:
                nc.vector.transpose(cpblk[0:32, 0:32], cellblk[:])

        LAG = 4  # scan step t emitted after frame t+LAG-1's conv
        for t in range(T):
            if t >= LAG:
                emit_scan(t - LAG)
            emit_conv(t)
        for t in range(T - LAG, T):
            emit_scan(t)

        # tail: h_23 = c_23 * tanh(c_23) = c_23 * (2*sig(2*c_23) - 1)
        s4f = tmp.tile([1, U], FP32, tag='s4f')
        nc.scalar.activation(s4f[:], cellv, Sigmoid, scale=2.0)
        h4f = tmp.tile([1, U], FP32, tag='h4f')
        nc.gpsimd.tensor_add(h4f[:], s4f[:], s4f[:])
        nc.gpsimd.tensor_sub(h4f[:], h4f[:], onesc[:])
        hout = tmp.tile([1, U], FP32, tag='hout')
        nc.gpsimd.tensor_mul(hout[:], h4f[:], cellv)
        nc.sync.dma_start(outh_d[:], hout[:])

        for p in (scrb, scra, tmp, ga_pool, zb_pool, gs, spsum, psum,
                  stackp, state, const):
            p.release()

    return nc


# -------------------------------------------------------------- host prep
def _prep_inputs(x, conv_w, conv_b, W1, b1, W2, b2, W3, b3):
    x = np.asarray(x, np.float32)
    conv_w = np.asarray(conv_w, np.float32)
    conv_b = np.asarray(conv_b, np.float32)

    # stack [B, T, 90, 14*112]: p = (3c+dx)*10 + wr -> xpad[8jb+wr, x+dx, c]
    xp = np.zeros((B, T, H + 2, W + 2, C), np.float32)
    xp[:, :, 1:H + 1, 1:W + 1, :] = x
    xin3 = np.empty((B, T, KP, NB, W), np.float32)
    rows = 8 * np.arange(NB)
    for c in range(3):
        for dx in range(3):
            for wr in range(WR):
                p = (3 * c + dx) * WR + wr
                xin3[:, :, p] = np.moveaxis(
                    xp[:, :, rows + wr, dx:dx + W, c], 0, 2)
    xin3 = xin3.reshape(B, T, KP, FREE).astype(ml_dtypes.float8_e4m3)

    # stationary [3, 90, 128]: smat[g, p, 16i + fl] = w[wr-i, dx, c, 16g+fl]
    smat = np.zeros((G, KP, M), np.float32)
    for c in range(3):
        for dx in range(3):
            for i in range(P8):
                for dy in range(3):
                    wr = i + dy
                    p = (3 * c + dx) * WR + wr
                    for g in range(G):
                        smat[g, p, 16 * i:16 * i + 16] = \
                            conv_w[dy, dx, c, 16 * g:16 * g + 16]
    smat = smat.astype(ml_dtypes.bfloat16)

    # per-group conv bias, per partition 16i+fl
    cbias = np.empty((M, G), np.float32)
    for g in range(G):
        cbias[:, g] = np.tile(conv_b[16 * g:16 * g + 16], P8)

    # gate weights: cols [0:32]=W1, [32:64]=W2, [64:96]=2*W3 (tanh fold)
    Wcat = np.stack([np.asarray(W1, np.float32), np.asarray(W2, np.float32),
                     np.asarray(W3, np.float32)], axis=0)  # [3, 80, 32]
    scale = np.array([1.0, 1.0, 2.0], np.float32)
    # feat part: wfeat[128g+16i+fl -> wait: rows are (i, fl) of group g
    wfeat = np.zeros((M, G * 96), np.float32)
    for g in range(G):
        for gate in range(3):
            blk = Wcat[gate, 16 * g:16 * g + 16, :] * scale[gate] / float(H * W)
            # rows 16i+fl, cols 96g+32*gate+u : same 16-filter block per i
            wfeat[:, 96 * g + 32 * gate:96 * g + 32 * gate + 32] = \
                np.tile(blk, (P8, 1))
    wfeat = wfeat.astype(ml_dtypes.bfloat16)

    # hidden path + 2*I identity block: whidi [128, 128] (rows 32+ zero-pad
    # so the scan matmuls keep K=128 / (128,128) PE tiling mode)
    whidi = np.zeros((M, M), np.float32)
    for gate in range(3):
        whidi[0:U, 32 * gate:32 * gate + 32] = \
            Wcat[gate, F:F + U, :] * scale[gate]
    whidi[0:U, 96:128] = 2.0 * np.eye(U, dtype=np.float32)
    whidi = whidi.astype(ml_dtypes.bfloat16)

    gb = np.zeros((1, M), np.float32)
    gb[0, 0:32] = np.asarray(b1, np.float32)
    gb[0, 32:64] = np.asarray(b2, np.float32)
    gb[0, 64:96] = 2.0 * np.asarray(b3, np.float32)
    gbias = gb.astype(ml_dtypes.bfloat16)

    return xin3, smat, cbias, wfeat, whidi, gbias


# ------------------------------------------------------------------ kernel
def kernel(x, conv_w, conv_b, W1, b1, W2, b2, W3, b3, W4, b4):
    global LAST_RESULTS
    xin3, smat, cbias, wfeat, whidi, gbias = _prep_inputs(
        x, conv_w, conv_b, W1, b1, W2, b2, W3, b3)

    nc = _build_bass(
        use_gbias=bool(np.any(np.asarray(gbias, np.float32))),
        use_cbias=bool(np.any(np.asarray(conv_b, np.float32))))
    in_maps = [{
        'xin': np.ascontiguousarray(xin3[b]),
        'smat': smat,
        'cbias': cbias,
        'wfeat': wfeat,
        'whidi': whidi,
        'gbias': gbias,
    } for b in range(B)]

    res = run_bass_kernel_spmd(nc, in_maps, core_ids=list(range(B)))
    LAST_RESULTS = res
    out = np.stack([res.results[b]['outh'][0] for b in range(B)], axis=0)
    return out.astype(np.float32)
